# revision 4
# baseline (speedup 1.0000x reference)
"""MoE soft-routing MLP kernel for 8 Trainium2 NeuronCores.

Reference computation (per layer l, weights a_l: [E, out, in], bias b_l: [E, out]):
    y_e = H @ a_e^T + b_e          # per-expert GEMM      [B, out]
    H'  = sum_e wb[e, :, None] * y_e                      [B, out]
    H'  = elu(H') for layers 0, 1

Distribution: data-parallel over batch B=4096 across 8 cores (B_loc=512).
Expert weights are replicated to every core; x and weight_blend are sharded
along batch.

Per-core algorithm (activations kept TRANSPOSED on chip: [feature, batch]):
    out[o, b] = sum_e sum_i aT_e[i, o] * (wb[e, b] * Ht[i, b])  + bias blend
  - each expert's contribution accumulates into one PSUM bank per output
    chunk: lhsT = aT_e[i-tile, o-chunk] (128x128 stationary), rhs = zt_e =
    Ht[i-tile] * bcast(wb[e, :]) (128x512 moving, fp16),
  - ELU+1 is evicted as relu(x) + min(exp(x), 1) into fp32 SBUF; the -1
    folds into the next layer's blend: zt = (h - 1) * wbb_e (one DVE op).

Matmuls are fp16 with fp32 PSUM accumulation. Weights are pre-scaled by 2^8
and blend weights by 2^6 on the host so fp16 products stay clear of the
subnormal range; the 2^-14 descale folds into the PSUM-eviction scales.
Measured end-to-end max rel-err vs the fp32 reference: ~5e-4.

Performance model (measured on hw):
  - PE is the wall: 1024 matmuls x 512 rows = 524288 cycles ~ 215us at
    2.4GHz. fp8 DoubleRow runs at the same rows/cycle (2x MACs via the
    in-pair contraction) so the ~8-bit precision this problem needs
    (hi+lo fp8 on both operands = 3 GEMM terms) would cost 1.5x fp16 —
    fp8 does not pay here. f32r matches fp16 rate but doubles DMA.
  - DMA: one big contiguous dma_start sustains ~370 GB/s (16 SDMA engines);
    partition-splitting a transfer HALVES bulk bandwidth and small chunks
    pay ~2us completion latency each. So weights stream as one 1-2MB
    dma_start per (layer, expert) slab, host-packed partition-contiguous
    ([128, ni*dout] rows). 34MB total ~ 95us, fully hidden under PE.
  - Startup: x^T (fp16, scalar queue) + wbb[0] (gpsimd) + first weight
    slab (sync) land in ~3us while junk matmuls warm the PE HAM clock
    gate (~3.4us of activity to reach 2.4GHz).
  - Tail: the final layer stores fp16 banks with a single dma_start each
    (no 8-way splitting), alternating scalar/sync queues.

The output is DMA'd out transposed ([512, 512] fp16 per core) and
un-transposed + upcast on the host.
"""

import os
import sys

if "/opt/trn_rl_repo" not in sys.path:
    sys.path.insert(0, "/opt/trn_rl_repo")

import numpy as np

import concourse.bass as bass  # noqa: F401  (bass must import before mybir use)
import concourse.mybir as mybir
import concourse.tile as tile
from concourse import bacc
from concourse.bass_utils import run_bass_kernel_spmd

F32 = mybir.dt.float32
F16 = mybir.dt.float16
AF = mybir.ActivationFunctionType
ALU = mybir.AluOpType

WEXP, ZEXP = 8, 6
DESCALE = float(2.0 ** -(WEXP + ZEXP))

B, E = 4096, 8
DIMS = [512, 1024, 1024, 512]
N_CORES = 8
B_LOC = B // N_CORES  # 512; also the matmul moving free-dim (max for 4-byte)
P = 128

# (in, out, apply_elu) per layer
LAYERS = [
    (DIMS[0], DIMS[1], True),
    (DIMS[1], DIMS[2], True),
    (DIMS[2], DIMS[3], False),
]

LAST_RESULTS = None  # BassKernelResults of the most recent run (for test.py)
_NC_CACHE = {}


def _build(has_bias):
    """Build the per-core module. has_bias=False (the case this problem's
    setup_inputs actually produces — all beta fills are zeros) drops the
    blended-bias matmuls and their beta/wb feeds entirely; each bank then
    closes on the last expert's product."""
    nc = bacc.Bacc(None, target_bir_lowering=False, debug=False)

    # xt host-packed [128, ni0, B_LOC] fp16: (p, j, b) = x^T[j*128+p, b]
    ni0 = DIMS[0] // P
    xt = nc.dram_tensor("xt", [P, ni0, B_LOC], F16, kind="ExternalInput")
    # wbb host-packed [128, E, B_LOC] fp32 (partition-broadcast blend weights)
    wbbd = nc.dram_tensor("wbb", [P, E, B_LOC], F32, kind="ExternalInput")
    # weights host-packed per layer: [E, 128, ni, dout] fp16,
    # (e, p, j, o) = aT_l[e, j*128+p, o] — each expert slab is one
    # partition-contiguous [128, ni*dout] DMA.
    ats = [
        nc.dram_tensor(f"a{l}t", [E, P, din // P, dout], F16, kind="ExternalInput")
        for l, (din, dout, _) in enumerate(LAYERS)
    ]
    wb, betas = None, []
    if has_bias:
        wb = nc.dram_tensor("wb", [E, B_LOC], F16, kind="ExternalInput")
        betas = [
            nc.dram_tensor(f"b{l}", [E, dout], F16, kind="ExternalInput")
            for l, (_, dout, _) in enumerate(LAYERS)
        ]
    outt = nc.dram_tensor("outt", [DIMS[3], B_LOC], F16, kind="ExternalOutput")

    with tile.TileContext(nc) as tc:
        with (
            tc.tile_pool(name="xp", bufs=1) as xp,
            tc.tile_pool(name="htp", bufs=12) as htp,
            tc.tile_pool(name="ztp", bufs=16) as ztp,
            tc.tile_pool(name="wp", bufs=3) as wp,
            tc.tile_pool(name="wbbp", bufs=1) as wbbp,
            tc.tile_pool(name="consts", bufs=1) as consts,
            tc.tile_pool(name="betap", bufs=2) as betap,
            tc.tile_pool(name="tmp", bufs=4) as tmp,
            tc.tile_pool(name="psp", bufs=8, space="PSUM") as psp,
        ):
            # --- startup ---
            # PE warm-up: the HAM clock gate needs ~3.4us of sustained PE
            # activity to reach 2.4 GHz; junk matmuls also cover the first
            # input DMAs (~3us).
            junk = consts.tile([P, B_LOC], F16, tag="junk")
            nc.vector.memset(junk, 0.0)
            warm_ps = psp.tile([P, B_LOC], F32, tag="ps")
            for _ in range(10):
                nc.tensor.matmul(warm_ps, junk[:, :P], junk, start=True, stop=True)

            # Startup feeds, all on the scalar HWDGE queue (fast ~0.6us
            # first-byte; gpsimd SWDGE would add ~2us fixed + ~3us
            # end-of-kernel drain). Separate tiles for wbb[0] vs wbb[1:] so
            # the first blend doesn't wait on the bulk wbb transfer (Tile
            # tracks dependencies per tile, not per region).
            wbb0_sb = wbbp.tile([P, 1, B_LOC], F32, tag="wbb0")
            nc.scalar.dma_start(out=wbb0_sb, in_=wbbd[:, 0:1, :])
            xt_sb = xp.tile([P, ni0, B_LOC], F16, tag="xt")
            nc.scalar.dma_start(out=xt_sb, in_=xt[:, :, :])
            wbbr_sb = wbbp.tile([P, E - 1, B_LOC], F32, tag="wbbr")
            nc.scalar.dma_start(out=wbbr_sb, in_=wbbd[:, 1:E, :])
            wbb = [wbb0_sb[:, 0, :]] + [wbbr_sb[:, e - 1, :] for e in range(1, E)]
            # wb as [E, B_LOC] tile: rhs of the bias matmuls
            wb_all = None
            if has_bias:
                wb_all = consts.tile([E, B_LOC], F16, tag="wb_all")
                nc.gpsimd.dma_start(out=wb_all, in_=wb[:, :])

            # --- layers ---
            ht = [xt_sb[:, j, :] for j in range(ni0)]
            for l, (din, dout, use_act) in enumerate(LAYERS):
                ni, no = din // P, dout // P
                beta_sb = None
                if has_bias:
                    beta_sb = betap.tile([E, dout], F16, tag="beta")
                    nc.gpsimd.dma_start(out=beta_sb, in_=betas[l][:, :])

                psums = [psp.tile([P, B_LOC], F32, tag="ps", name="ps") for _ in range(no)]

                # one partition-contiguous weight slab per expert on the
                # sync queue (~370GB/s; stays well ahead of PE consumption)
                def load_slab(e):
                    t = wp.tile([P, ni, dout], F16, tag=f"w{l}")
                    nc.sync.dma_start(out=t, in_=ats[l][e])
                    return t

                slabs = {0: load_slab(0), 1: load_slab(1)}

                # experts 0..E-2 j-outer (consumes ht tiles as the previous
                # layer produces them; first expert opens each bank)
                for e in range(E - 1):
                    at_sb = slabs.pop(e)
                    if e + 2 < E:
                        slabs[e + 2] = load_slab(e + 2)
                    for j in range(ni):
                        zt = ztp.tile([P, B_LOC], F16, tag="zt")
                        if l == 0:
                            nc.vector.tensor_mul(zt, ht[j], wbb[e])
                        else:
                            # ht holds elu(x)+1; fold the -1 into the blend
                            nc.vector.scalar_tensor_tensor(
                                zt, ht[j], -1.0, wbb[e], ALU.add, ALU.mult
                            )
                        for c in range(no):
                            nc.tensor.matmul(
                                psums[c],
                                at_sb[:, j, c * P : (c + 1) * P],
                                zt,
                                start=(e == 0 and j == 0),
                                stop=False,
                            )
                # last expert runs c-outer (bank-by-bank) so bank closures —
                # and therefore evictions, next-layer bank reuse, and the
                # final output stores — spread across the last ~ni*no
                # matmuls instead of clustering after the end.
                e = E - 1
                at_sb = slabs.pop(e)
                zts = []
                for j in range(ni):
                    zt = ztp.tile([P, B_LOC], F16, tag="zt")
                    if l == 0:
                        nc.vector.tensor_mul(zt, ht[j], wbb[e])
                    else:
                        nc.vector.scalar_tensor_tensor(
                            zt, ht[j], -1.0, wbb[e], ALU.add, ALU.mult
                        )
                    zts.append(zt)
                new_ht = []
                for c in range(no):
                    for j in range(ni):
                        nc.tensor.matmul(
                            psums[c],
                            at_sb[:, j, c * P : (c + 1) * P],
                            zts[j],
                            start=False,
                            stop=(not has_bias and j == ni - 1),
                        )
                    if has_bias:
                        nc.tensor.matmul(
                            psums[c],
                            beta_sb[:, c * P : (c + 1) * P],
                            wb_all,
                            start=False,
                            stop=True,
                        )

                    # evict bank c as soon as it closes:
                    # elu(x)+1 into fp32 ht for layers 0/1, fp16 DMA out for
                    # layer 2
                    if use_act:
                        r = tmp.tile([P, B_LOC], F32, tag="relu")
                        x = tmp.tile([P, B_LOC], F32, tag="expz")
                        h = htp.tile([P, B_LOC], F32, tag="ht")
                        nc.scalar.activation(r, psums[c], AF.Relu, scale=DESCALE)
                        nc.scalar.activation(x, psums[c], AF.Exp, scale=DESCALE)
                        # h = min(x, 1) + r  ( = elu + 1 )
                        nc.vector.scalar_tensor_tensor(h, x, 1.0, r, ALU.min, ALU.add)
                        new_ht.append(h)
                    else:
                        # descale-copy split across ACT and DVE (engine time
                        # scales with the free dim; both engines are
                        # otherwise idle at kernel end), then one unsplit
                        # 128KB store per bank on an alternating HWDGE queue
                        o = tmp.tile([P, B_LOC], F16, tag="out")
                        nc.scalar.activation(
                            o[: P // 2, :], psums[c][: P // 2, :], AF.Copy,
                            scale=DESCALE,
                        )
                        nc.vector.tensor_scalar_mul(
                            o[P // 2 :, :], psums[c][P // 2 :, :], DESCALE
                        )
                        eng = nc.scalar if c % 2 == 0 else nc.sync
                        eng.dma_start(
                            out=outt[c * P : (c + 1) * P, :], in_=o
                        )
                ht = new_ht

    nc.compile()
    return nc


def _maybe_reset_device():
    """Clear stale NRT state on the axon terminal left by a crashed prior
    process. Only safe/needed before this process initializes its jax
    backend, and must run in a subprocess (CDLL'ing the axon .so in-process
    conflicts with jax's own dlopen)."""
    try:
        import jax._src.xla_bridge as xb

        if getattr(xb, "_backends", None):
            return  # backend already live in this process; don't touch it
    except Exception:
        pass
    try:
        import subprocess

        subprocess.run(
            [
                sys.executable,
                "-c",
                "import ctypes; lib = ctypes.CDLL('/opt/axon/libaxon_pjrt.so'); "
                "lib.axon_reset.restype = ctypes.c_int64; lib.axon_reset()",
            ],
            timeout=60,
            capture_output=True,
        )
    except Exception:
        pass


def kernel(x, weight_blend, a0, b0, a1, b1, a2, b2):
    global LAST_RESULTS, _NC_CACHE
    _maybe_reset_device()
    x = np.asarray(x, dtype=np.float32)
    weight_blend = np.ascontiguousarray(np.asarray(weight_blend, dtype=np.float32))
    aT = []
    for a, (din, dout, _) in zip((a0, a1, a2), LAYERS):
        # [E, dout, din] -> aT [E, din, dout] -> [E, ni, 128, dout]
        # -> [E, 128, ni, dout] so each expert slab is one
        # partition-contiguous DMA
        at = (np.asarray(a, dtype=np.float32) * float(2.0**WEXP)).transpose(0, 2, 1)
        at = at.reshape(E, din // P, P, dout).transpose(0, 2, 1, 3)
        aT.append(np.ascontiguousarray(at.astype(np.float16)))
    bs = [
        np.ascontiguousarray(
            (np.asarray(b, dtype=np.float32) * float(2.0 ** (WEXP + ZEXP))).astype(
                np.float16
            )
        )
        for b in (b0, b1, b2)
    ]
    has_bias = any(np.any(b) for b in bs)

    if has_bias not in _NC_CACHE:
        _NC_CACHE[has_bias] = _build(has_bias)
    nc = _NC_CACHE[has_bias]

    in_maps = []
    for c in range(N_CORES):
        sl = slice(c * B_LOC, (c + 1) * B_LOC)
        wb_c = np.ascontiguousarray(weight_blend[:, sl]) * float(2.0**ZEXP)
        xt_c = x[sl].T.reshape(DIMS[0] // P, P, B_LOC).transpose(1, 0, 2)
        wbb_c = np.broadcast_to(wb_c[None, :, :], (P, E, B_LOC))
        m = {
            "xt": np.ascontiguousarray(xt_c.astype(np.float16)),
            "wbb": np.ascontiguousarray(wbb_c),
            "a0t": aT[0],
            "a1t": aT[1],
            "a2t": aT[2],
        }
        if has_bias:
            m["wb"] = wb_c.astype(np.float16)
            m["b0"], m["b1"], m["b2"] = bs
        in_maps.append(m)

    trace = os.environ.get("BASS_KERNEL_TRACE") == "1"
    res = run_bass_kernel_spmd(
        nc, in_maps, core_ids=list(range(N_CORES)), trace=trace
    )
    LAST_RESULTS = res
    return np.concatenate(
        [np.asarray(r["outt"]).T.astype(np.float32) for r in res.results], axis=0
    )


# revision 6
# speedup vs baseline: 1.0163x; 1.0163x over previous
"""MoE soft-routing MLP kernel for 8 Trainium2 NeuronCores.

Reference computation (per layer l, weights a_l: [E, out, in], bias b_l: [E, out]):
    y_e = H @ a_e^T + b_e          # per-expert GEMM      [B, out]
    H'  = sum_e wb[e, :, None] * y_e                      [B, out]
    H'  = elu(H') for layers 0, 1

Distribution: data-parallel over batch B=4096 across 8 cores (B_loc=512).
Expert weights are replicated to every core; x and weight_blend are sharded
along batch.

Per-core algorithm (activations kept TRANSPOSED on chip: [feature, batch]):
    out[o, b] = sum_e sum_i aT_e[i, o] * (wb[e, b] * Ht[i, b])  + bias blend
  - each expert's contribution accumulates into one PSUM bank per output
    chunk: lhsT = aT_e[i-tile, o-chunk] (128x128 stationary), rhs = zt_e =
    Ht[i-tile] * bcast(wb[e, :]) (128x512 moving, fp16),
  - ELU+1 is evicted as relu(x) + min(exp(x), 1) into fp32 SBUF; the -1
    folds into the next layer's blend: zt = (h - 1) * wbb_e (one DVE op).

Matmuls are fp16 with fp32 PSUM accumulation. Weights are pre-scaled by 2^8
and blend weights by 2^6 on the host so fp16 products stay clear of the
subnormal range; the 2^-14 descale folds into the PSUM-eviction scales.
Measured end-to-end max rel-err vs the fp32 reference: ~5e-4.

Performance model (measured on hw):
  - PE is the wall: 1024 matmuls x 512 rows = 524288 cycles ~ 215us at
    2.4GHz. fp8 DoubleRow runs at the same rows/cycle (2x MACs via the
    in-pair contraction) so the ~8-bit precision this problem needs
    (hi+lo fp8 on both operands = 3 GEMM terms) would cost 1.5x fp16 —
    fp8 does not pay here. f32r matches fp16 rate but doubles DMA.
  - DMA: one big contiguous dma_start sustains ~370 GB/s (16 SDMA engines);
    partition-splitting a transfer HALVES bulk bandwidth and small chunks
    pay ~2us completion latency each. So weights stream as one 1-2MB
    dma_start per (layer, expert) slab, host-packed partition-contiguous
    ([128, ni*dout] rows). 34MB total ~ 95us, fully hidden under PE.
  - Startup: x^T (fp16, scalar queue) + wbb[0] (gpsimd) + first weight
    slab (sync) land in ~3us while junk matmuls warm the PE HAM clock
    gate (~3.4us of activity to reach 2.4GHz).
  - Tail: the final layer stores fp16 banks with a single dma_start each
    (no 8-way splitting), alternating scalar/sync queues.

The output is DMA'd out transposed ([512, 512] fp16 per core) and
un-transposed + upcast on the host.
"""

import os
import sys

if "/opt/trn_rl_repo" not in sys.path:
    sys.path.insert(0, "/opt/trn_rl_repo")

import numpy as np

import concourse.bass as bass  # noqa: F401  (bass must import before mybir use)
import concourse.mybir as mybir
import concourse.tile as tile
from concourse import bacc
from concourse.bass_utils import run_bass_kernel_spmd

F32 = mybir.dt.float32
F16 = mybir.dt.float16
AF = mybir.ActivationFunctionType
ALU = mybir.AluOpType

WEXP, ZEXP = 8, 6
DESCALE = float(2.0 ** -(WEXP + ZEXP))

B, E = 4096, 8
DIMS = [512, 1024, 1024, 512]
N_CORES = 8
B_LOC = B // N_CORES  # 512; also the matmul moving free-dim (max for 4-byte)
P = 128

# (in, out, apply_elu) per layer
LAYERS = [
    (DIMS[0], DIMS[1], True),
    (DIMS[1], DIMS[2], True),
    (DIMS[2], DIMS[3], False),
]

LAST_RESULTS = None  # BassKernelResults of the most recent run (for test.py)
_NC_CACHE = {}


def _build(has_bias):
    """Build the per-core module. has_bias=False (the case this problem's
    setup_inputs actually produces — all beta fills are zeros) drops the
    blended-bias matmuls and their beta/wb feeds entirely; each bank then
    closes on the last expert's product."""
    nc = bacc.Bacc(None, target_bir_lowering=False, debug=False)

    # xt host-packed [128, ni0, B_LOC] fp16: (p, j, b) = x^T[j*128+p, b]
    ni0 = DIMS[0] // P
    xt = nc.dram_tensor("xt", [P, ni0, B_LOC], F16, kind="ExternalInput")
    # wbb host-packed [128, E, B_LOC] fp32 (partition-broadcast blend weights)
    wbbd = nc.dram_tensor("wbb", [P, E, B_LOC], F32, kind="ExternalInput")
    # weights host-packed per layer: [E, 128, ni, dout] fp16,
    # (e, p, j, o) = aT_l[e, j*128+p, o] — each expert slab is one
    # partition-contiguous [128, ni*dout] DMA.
    ats = [
        nc.dram_tensor(f"a{l}t", [E, P, din // P, dout], F16, kind="ExternalInput")
        for l, (din, dout, _) in enumerate(LAYERS)
    ]
    wb, betas = None, []
    if has_bias:
        wb = nc.dram_tensor("wb", [E, B_LOC], F16, kind="ExternalInput")
        betas = [
            nc.dram_tensor(f"b{l}", [E, dout], F16, kind="ExternalInput")
            for l, (_, dout, _) in enumerate(LAYERS)
        ]
    outt = nc.dram_tensor("outt", [DIMS[3], B_LOC], F16, kind="ExternalOutput")

    with tile.TileContext(nc) as tc:
        with (
            tc.tile_pool(name="xp", bufs=1) as xp,
            tc.tile_pool(name="htp", bufs=12) as htp,
            tc.tile_pool(name="ztp", bufs=16) as ztp,
            tc.tile_pool(name="wp", bufs=3) as wp,
            tc.tile_pool(name="wbbp", bufs=1) as wbbp,
            tc.tile_pool(name="consts", bufs=1) as consts,
            tc.tile_pool(name="betap", bufs=2) as betap,
            tc.tile_pool(name="tmp", bufs=4) as tmp,
            tc.tile_pool(name="psp", bufs=8, space="PSUM") as psp,
        ):
            # --- startup ---
            # PE warm-up: the HAM clock gate needs ~3.4us of sustained PE
            # activity to reach 2.4 GHz; junk matmuls also cover the first
            # input DMAs (~3us).
            junk = consts.tile([P, B_LOC], F16, tag="junk")
            nc.vector.memset(junk, 0.0)
            warm_ps = psp.tile([P, B_LOC], F32, tag="ps")
            for _ in range(10):
                nc.tensor.matmul(warm_ps, junk[:, :P], junk, start=True, stop=True)

            # Startup feeds share the single sync HWDGE queue with the
            # weight-slab stream, in priority order (wbb0, xt, slab0, wbbR,
            # slab1, ...): a second HWDGE ring gets starved for ~4us while
            # the first one has a packet backlog, so splitting queues
            # delays, not accelerates, the critical path. Separate tiles
            # for wbb[0] vs wbb[1:] so the first blend doesn't wait on the
            # bulk wbb transfer (Tile tracks dependencies per tile).
            wbb0_sb = wbbp.tile([P, 1, B_LOC], F32, tag="wbb0")
            nc.sync.dma_start(out=wbb0_sb, in_=wbbd[:, 0:1, :])
            xt_sb = xp.tile([P, ni0, B_LOC], F16, tag="xt")
            nc.sync.dma_start(out=xt_sb, in_=xt[:, :, :])
            wbbr_sb = wbbp.tile([P, E - 1, B_LOC], F32, tag="wbbr")
            wbb = [wbb0_sb[:, 0, :]] + [wbbr_sb[:, e - 1, :] for e in range(1, E)]
            # wb as [E, B_LOC] tile: rhs of the bias matmuls
            wb_all = None
            if has_bias:
                wb_all = consts.tile([E, B_LOC], F16, tag="wb_all")
                nc.gpsimd.dma_start(out=wb_all, in_=wb[:, :])

            # --- layers ---
            ht = [xt_sb[:, j, :] for j in range(ni0)]
            for l, (din, dout, use_act) in enumerate(LAYERS):
                ni, no = din // P, dout // P
                beta_sb = None
                if has_bias:
                    beta_sb = betap.tile([E, dout], F16, tag="beta")
                    nc.gpsimd.dma_start(out=beta_sb, in_=betas[l][:, :])

                psums = [psp.tile([P, B_LOC], F32, tag="ps", name="ps") for _ in range(no)]

                # one partition-contiguous weight slab per expert on the
                # sync queue (~370GB/s; stays well ahead of PE consumption)
                def load_slab(e):
                    t = wp.tile([P, ni, dout], F16, tag=f"w{l}")
                    nc.sync.dma_start(out=t, in_=ats[l][e])
                    return t

                slabs = {0: load_slab(0)}
                if l == 0:
                    # bulk blend weights ride after the first slab; needed
                    # from expert 1 (~7us after the first real matmul)
                    nc.sync.dma_start(out=wbbr_sb, in_=wbbd[:, 1:E, :])
                slabs[1] = load_slab(1)

                # experts 0..E-2 j-outer (consumes ht tiles as the previous
                # layer produces them; first expert opens each bank)
                for e in range(E - 1):
                    at_sb = slabs.pop(e)
                    if e + 2 < E:
                        slabs[e + 2] = load_slab(e + 2)
                    for j in range(ni):
                        zt = ztp.tile([P, B_LOC], F16, tag="zt")
                        if l == 0:
                            nc.vector.tensor_mul(zt, ht[j], wbb[e])
                        else:
                            # ht holds elu(x)+1; fold the -1 into the blend
                            nc.vector.scalar_tensor_tensor(
                                zt, ht[j], -1.0, wbb[e], ALU.add, ALU.mult
                            )
                        for c in range(no):
                            nc.tensor.matmul(
                                psums[c],
                                at_sb[:, j, c * P : (c + 1) * P],
                                zt,
                                start=(e == 0 and j == 0),
                                stop=False,
                            )
                # last expert runs c-outer (bank-by-bank) so bank closures —
                # and therefore evictions, next-layer bank reuse, and the
                # final output stores — spread across the last ~ni*no
                # matmuls instead of clustering after the end.
                e = E - 1
                at_sb = slabs.pop(e)
                zts = []
                for j in range(ni):
                    zt = ztp.tile([P, B_LOC], F16, tag="zt")
                    if l == 0:
                        nc.vector.tensor_mul(zt, ht[j], wbb[e])
                    else:
                        nc.vector.scalar_tensor_tensor(
                            zt, ht[j], -1.0, wbb[e], ALU.add, ALU.mult
                        )
                    zts.append(zt)
                new_ht = []
                for c in range(no):
                    for j in range(ni):
                        nc.tensor.matmul(
                            psums[c],
                            at_sb[:, j, c * P : (c + 1) * P],
                            zts[j],
                            start=False,
                            stop=(not has_bias and j == ni - 1),
                        )
                    if has_bias:
                        nc.tensor.matmul(
                            psums[c],
                            beta_sb[:, c * P : (c + 1) * P],
                            wb_all,
                            start=False,
                            stop=True,
                        )

                    # evict bank c as soon as it closes:
                    # elu(x)+1 into fp32 ht for layers 0/1, fp16 DMA out for
                    # layer 2
                    if use_act:
                        r = tmp.tile([P, B_LOC], F32, tag="relu")
                        x = tmp.tile([P, B_LOC], F32, tag="expz")
                        h = htp.tile([P, B_LOC], F32, tag="ht")
                        nc.scalar.activation(r, psums[c], AF.Relu, scale=DESCALE)
                        nc.scalar.activation(x, psums[c], AF.Exp, scale=DESCALE)
                        # h = min(x, 1) + r  ( = elu + 1 )
                        nc.vector.scalar_tensor_tensor(h, x, 1.0, r, ALU.min, ALU.add)
                        new_ht.append(h)
                    else:
                        # descale-copy split across ACT and DVE (engine time
                        # scales with the free dim; both engines are
                        # otherwise idle at kernel end), then one unsplit
                        # 128KB store per bank on an alternating HWDGE queue
                        o = tmp.tile([P, B_LOC], F16, tag="out")
                        nc.scalar.activation(
                            o[: P // 2, :], psums[c][: P // 2, :], AF.Copy,
                            scale=DESCALE,
                        )
                        nc.vector.tensor_scalar_mul(
                            o[P // 2 :, :], psums[c][P // 2 :, :], DESCALE
                        )
                        eng = nc.scalar if c % 2 == 0 else nc.sync
                        eng.dma_start(
                            out=outt[c * P : (c + 1) * P, :], in_=o
                        )
                ht = new_ht

    nc.compile()
    return nc


def _maybe_reset_device():
    """Clear stale NRT state on the axon terminal left by a crashed prior
    process. Only safe/needed before this process initializes its jax
    backend, and must run in a subprocess (CDLL'ing the axon .so in-process
    conflicts with jax's own dlopen)."""
    try:
        import jax._src.xla_bridge as xb

        if getattr(xb, "_backends", None):
            return  # backend already live in this process; don't touch it
    except Exception:
        pass
    try:
        import subprocess

        subprocess.run(
            [
                sys.executable,
                "-c",
                "import ctypes; lib = ctypes.CDLL('/opt/axon/libaxon_pjrt.so'); "
                "lib.axon_reset.restype = ctypes.c_int64; lib.axon_reset()",
            ],
            timeout=60,
            capture_output=True,
        )
    except Exception:
        pass


def kernel(x, weight_blend, a0, b0, a1, b1, a2, b2):
    global LAST_RESULTS, _NC_CACHE
    _maybe_reset_device()
    x = np.asarray(x, dtype=np.float32)
    weight_blend = np.ascontiguousarray(np.asarray(weight_blend, dtype=np.float32))
    aT = []
    for a, (din, dout, _) in zip((a0, a1, a2), LAYERS):
        # [E, dout, din] -> aT [E, din, dout] -> [E, ni, 128, dout]
        # -> [E, 128, ni, dout] so each expert slab is one
        # partition-contiguous DMA
        at = (np.asarray(a, dtype=np.float32) * float(2.0**WEXP)).transpose(0, 2, 1)
        at = at.reshape(E, din // P, P, dout).transpose(0, 2, 1, 3)
        aT.append(np.ascontiguousarray(at.astype(np.float16)))
    bs = [
        np.ascontiguousarray(
            (np.asarray(b, dtype=np.float32) * float(2.0 ** (WEXP + ZEXP))).astype(
                np.float16
            )
        )
        for b in (b0, b1, b2)
    ]
    has_bias = any(np.any(b) for b in bs)

    if has_bias not in _NC_CACHE:
        _NC_CACHE[has_bias] = _build(has_bias)
    nc = _NC_CACHE[has_bias]

    in_maps = []
    for c in range(N_CORES):
        sl = slice(c * B_LOC, (c + 1) * B_LOC)
        wb_c = np.ascontiguousarray(weight_blend[:, sl]) * float(2.0**ZEXP)
        xt_c = x[sl].T.reshape(DIMS[0] // P, P, B_LOC).transpose(1, 0, 2)
        wbb_c = np.broadcast_to(wb_c[None, :, :], (P, E, B_LOC))
        m = {
            "xt": np.ascontiguousarray(xt_c.astype(np.float16)),
            "wbb": np.ascontiguousarray(wbb_c),
            "a0t": aT[0],
            "a1t": aT[1],
            "a2t": aT[2],
        }
        if has_bias:
            m["wb"] = wb_c.astype(np.float16)
            m["b0"], m["b1"], m["b2"] = bs
        in_maps.append(m)

    trace = os.environ.get("BASS_KERNEL_TRACE") == "1"
    res = run_bass_kernel_spmd(
        nc, in_maps, core_ids=list(range(N_CORES)), trace=trace
    )
    LAST_RESULTS = res
    return np.concatenate(
        [np.asarray(r["outt"]).T.astype(np.float32) for r in res.results], axis=0
    )


# revision 7
# speedup vs baseline: 1.0349x; 1.0184x over previous
"""MoE soft-routing MLP kernel for 8 Trainium2 NeuronCores.

Reference computation (per layer l, weights a_l: [E, out, in], bias b_l: [E, out]):
    y_e = H @ a_e^T + b_e          # per-expert GEMM      [B, out]
    H'  = sum_e wb[e, :, None] * y_e                      [B, out]
    H'  = elu(H') for layers 0, 1

Distribution: data-parallel over batch B=4096 across 8 cores (B_loc=512).
Expert weights are replicated to every core; x and weight_blend are sharded
along batch.

Per-core algorithm (activations kept TRANSPOSED on chip: [feature, batch]):
    out[o, b] = sum_e sum_i aT_e[i, o] * (wb[e, b] * Ht[i, b])  + bias blend
  - each expert's contribution accumulates into one PSUM bank per output
    chunk: lhsT = aT_e[i-tile, o-chunk] (128x128 stationary), rhs = zt_e =
    Ht[i-tile] * bcast(wb[e, :]) (128x512 moving, fp16),
  - ELU+1 is evicted as relu(x) + min(exp(x), 1) into fp32 SBUF; the -1
    folds into the next layer's blend: zt = (h - 1) * wbb_e (one DVE op).

Matmuls are fp16 with fp32 PSUM accumulation. Weights are pre-scaled by 2^8
and blend weights by 2^6 on the host so fp16 products stay clear of the
subnormal range; the 2^-14 descale folds into the PSUM-eviction scales.
Measured end-to-end max rel-err vs the fp32 reference: ~5e-4.

Performance model (measured on hw):
  - PE is the wall: 1024 matmuls x 512 rows = 524288 cycles ~ 215us at
    2.4GHz. fp8 DoubleRow runs at the same rows/cycle (2x MACs via the
    in-pair contraction) so the ~8-bit precision this problem needs
    (hi+lo fp8 on both operands = 3 GEMM terms) would cost 1.5x fp16 —
    fp8 does not pay here. f32r matches fp16 rate but doubles DMA.
  - DMA: one big contiguous dma_start sustains ~370 GB/s (16 SDMA engines);
    partition-splitting a transfer HALVES bulk bandwidth and small chunks
    pay ~2us completion latency each. So weights stream as one 1-2MB
    dma_start per (layer, expert) slab, host-packed partition-contiguous
    ([128, ni*dout] rows). 34MB total ~ 95us, fully hidden under PE.
  - Startup: x^T (fp16, scalar queue) + wbb[0] (gpsimd) + first weight
    slab (sync) land in ~3us while junk matmuls warm the PE HAM clock
    gate (~3.4us of activity to reach 2.4GHz).
  - Tail: the final layer stores fp16 banks with a single dma_start each
    (no 8-way splitting), alternating scalar/sync queues.

The output is DMA'd out transposed ([512, 512] fp16 per core) and
un-transposed + upcast on the host.
"""

import os
import sys

if "/opt/trn_rl_repo" not in sys.path:
    sys.path.insert(0, "/opt/trn_rl_repo")

import numpy as np

import concourse.bass as bass  # noqa: F401  (bass must import before mybir use)
import concourse.mybir as mybir
import concourse.tile as tile
from concourse import bacc
from concourse.bass_utils import run_bass_kernel_spmd

F32 = mybir.dt.float32
F16 = mybir.dt.float16
AF = mybir.ActivationFunctionType
ALU = mybir.AluOpType

WEXP, ZEXP = 8, 6
DESCALE = float(2.0 ** -(WEXP + ZEXP))

B, E = 4096, 8
DIMS = [512, 1024, 1024, 512]
N_CORES = 8
B_LOC = B // N_CORES  # 512; also the matmul moving free-dim (max for 4-byte)
P = 128

# (in, out, apply_elu) per layer
LAYERS = [
    (DIMS[0], DIMS[1], True),
    (DIMS[1], DIMS[2], True),
    (DIMS[2], DIMS[3], False),
]

LAST_RESULTS = None  # BassKernelResults of the most recent run (for test.py)
_NC_CACHE = {}


def _build(has_bias):
    """Build the per-core module. has_bias=False (the case this problem's
    setup_inputs actually produces — all beta fills are zeros) drops the
    blended-bias matmuls and their beta/wb feeds entirely; each bank then
    closes on the last expert's product."""
    nc = bacc.Bacc(None, target_bir_lowering=False, debug=False)

    # xt host-packed [128, ni0, B_LOC] fp16: (p, j, b) = x^T[j*128+p, b]
    ni0 = DIMS[0] // P
    xt = nc.dram_tensor("xt", [P, ni0, B_LOC], F16, kind="ExternalInput")
    # wbb host-packed [128, E, B_LOC] fp16 (partition-broadcast blend weights)
    wbbd = nc.dram_tensor("wbb", [P, E, B_LOC], F16, kind="ExternalInput")
    # weights host-packed per layer: [E, 128, ni, dout] fp16,
    # (e, p, j, o) = aT_l[e, j*128+p, o] — each expert slab is one
    # partition-contiguous [128, ni*dout] DMA.
    ats = [
        nc.dram_tensor(f"a{l}t", [E, P, din // P, dout], F16, kind="ExternalInput")
        for l, (din, dout, _) in enumerate(LAYERS)
    ]
    wb, betas = None, []
    if has_bias:
        wb = nc.dram_tensor("wb", [E, B_LOC], F16, kind="ExternalInput")
        betas = [
            nc.dram_tensor(f"b{l}", [E, dout], F16, kind="ExternalInput")
            for l, (_, dout, _) in enumerate(LAYERS)
        ]
    outt = nc.dram_tensor("outt", [DIMS[3], B_LOC], F16, kind="ExternalOutput")

    with tile.TileContext(nc) as tc:
        with (
            tc.tile_pool(name="xp", bufs=1) as xp,
            tc.tile_pool(name="htp", bufs=12) as htp,
            tc.tile_pool(name="ztp", bufs=16) as ztp,
            tc.tile_pool(name="wp", bufs=3) as wp,
            tc.tile_pool(name="wbbp", bufs=1) as wbbp,
            tc.tile_pool(name="consts", bufs=1) as consts,
            tc.tile_pool(name="betap", bufs=2) as betap,
            tc.tile_pool(name="tmp", bufs=4) as tmp,
            tc.tile_pool(name="psp", bufs=8, space="PSUM") as psp,
        ):
            # --- startup ---
            # PE warm-up: the HAM clock gate needs ~3.4us of sustained PE
            # activity to reach 2.4 GHz; junk matmuls also cover the first
            # input DMAs (~3us).
            junk = consts.tile([P, B_LOC], F16, tag="junk")
            nc.vector.memset(junk, 0.0)
            warm_ps = psp.tile([P, B_LOC], F32, tag="ps")
            for _ in range(10):
                nc.tensor.matmul(warm_ps, junk[:, :P], junk, start=True, stop=True)

            # Startup feeds share the single sync HWDGE queue with the
            # weight-slab stream, in priority order (wbb0, xt, slab0, wbbR,
            # slab1, ...): a second HWDGE ring gets starved for ~4us while
            # the first one has a packet backlog, so splitting queues
            # delays, not accelerates, the critical path. Separate tiles
            # for wbb[0] vs wbb[1:] so the first blend doesn't wait on the
            # bulk wbb transfer (Tile tracks dependencies per tile).
            wbb0_sb = wbbp.tile([P, 1, B_LOC], F16, tag="wbb0")
            nc.sync.dma_start(out=wbb0_sb, in_=wbbd[:, 0:1, :])
            xt_sb = xp.tile([P, ni0, B_LOC], F16, tag="xt")
            nc.sync.dma_start(out=xt_sb, in_=xt[:, :, :])
            wbbr_sb = wbbp.tile([P, E - 1, B_LOC], F16, tag="wbbr")
            wbb = [wbb0_sb[:, 0, :]] + [wbbr_sb[:, e - 1, :] for e in range(1, E)]
            # wb as [E, B_LOC] tile: rhs of the bias matmuls
            wb_all = None
            if has_bias:
                wb_all = consts.tile([E, B_LOC], F16, tag="wb_all")
                nc.gpsimd.dma_start(out=wb_all, in_=wb[:, :])

            # --- layers ---
            ht = [xt_sb[:, j, :] for j in range(ni0)]
            for l, (din, dout, use_act) in enumerate(LAYERS):
                ni, no = din // P, dout // P
                beta_sb = None
                if has_bias:
                    beta_sb = betap.tile([E, dout], F16, tag="beta")
                    nc.gpsimd.dma_start(out=beta_sb, in_=betas[l][:, :])

                psums = [psp.tile([P, B_LOC], F32, tag="ps", name="ps") for _ in range(no)]

                # one partition-contiguous weight slab per expert on the
                # sync queue (~370GB/s; stays well ahead of PE consumption)
                def load_slab(e, split_head=False):
                    t = wp.tile([P, ni, dout], F16, tag=f"w{l}")
                    if split_head:
                        nc.sync.dma_start(out=t[:, 0:1, :], in_=ats[l][e, :, 0:1, :])
                        nc.sync.dma_start(out=t[:, 1:ni, :], in_=ats[l][e, :, 1:ni, :])
                    else:
                        nc.sync.dma_start(out=t, in_=ats[l][e])
                    return t

                slabs = {0: load_slab(0, split_head=(l == 0))}
                if l == 0:
                    # bulk blend weights ride after the first slab; needed
                    # from expert 1 (~7us after the first real matmul)
                    nc.sync.dma_start(out=wbbr_sb, in_=wbbd[:, 1:E, :])
                slabs[1] = load_slab(1)

                # experts 0..E-2 j-outer (consumes ht tiles as the previous
                # layer produces them; first expert opens each bank)
                for e in range(E - 1):
                    at_sb = slabs.pop(e)
                    if e + 2 < E:
                        slabs[e + 2] = load_slab(e + 2)
                    for j in range(ni):
                        zt = ztp.tile([P, B_LOC], F16, tag="zt")
                        if l == 0:
                            nc.vector.tensor_mul(zt, ht[j], wbb[e])
                        else:
                            # ht holds elu(x)+1; fold the -1 into the blend
                            nc.vector.scalar_tensor_tensor(
                                zt, ht[j], -1.0, wbb[e], ALU.add, ALU.mult
                            )
                        for c in range(no):
                            nc.tensor.matmul(
                                psums[c],
                                at_sb[:, j, c * P : (c + 1) * P],
                                zt,
                                start=(e == 0 and j == 0),
                                stop=False,
                            )
                # last expert runs c-outer (bank-by-bank) so bank closures —
                # and therefore evictions, next-layer bank reuse, and the
                # final output stores — spread across the last ~ni*no
                # matmuls instead of clustering after the end.
                e = E - 1
                at_sb = slabs.pop(e)
                zts = []
                for j in range(ni):
                    zt = ztp.tile([P, B_LOC], F16, tag="zt")
                    if l == 0:
                        nc.vector.tensor_mul(zt, ht[j], wbb[e])
                    else:
                        nc.vector.scalar_tensor_tensor(
                            zt, ht[j], -1.0, wbb[e], ALU.add, ALU.mult
                        )
                    zts.append(zt)
                new_ht = []
                for c in range(no):
                    for j in range(ni):
                        nc.tensor.matmul(
                            psums[c],
                            at_sb[:, j, c * P : (c + 1) * P],
                            zts[j],
                            start=False,
                            stop=(not has_bias and j == ni - 1),
                        )
                    if has_bias:
                        nc.tensor.matmul(
                            psums[c],
                            beta_sb[:, c * P : (c + 1) * P],
                            wb_all,
                            start=False,
                            stop=True,
                        )

                    # evict bank c as soon as it closes:
                    # elu(x)+1 into fp32 ht for layers 0/1, fp16 DMA out for
                    # layer 2
                    if use_act:
                        r = tmp.tile([P, B_LOC], F32, tag="relu")
                        x = tmp.tile([P, B_LOC], F32, tag="expz")
                        h = htp.tile([P, B_LOC], F32, tag="ht")
                        nc.scalar.activation(r, psums[c], AF.Relu, scale=DESCALE)
                        nc.scalar.activation(x, psums[c], AF.Exp, scale=DESCALE)
                        # h = min(x, 1) + r  ( = elu + 1 )
                        nc.vector.scalar_tensor_tensor(h, x, 1.0, r, ALU.min, ALU.add)
                        new_ht.append(h)
                    else:
                        # descale-copy split across ACT and DVE (engine time
                        # scales with the free dim; both engines are
                        # otherwise idle at kernel end), then one unsplit
                        # 128KB store per bank on an alternating HWDGE queue
                        o = tmp.tile([P, B_LOC], F16, tag="out")
                        nc.scalar.activation(
                            o[: P // 2, :], psums[c][: P // 2, :], AF.Copy,
                            scale=DESCALE,
                        )
                        nc.vector.tensor_scalar_mul(
                            o[P // 2 :, :], psums[c][P // 2 :, :], DESCALE
                        )
                        eng = nc.scalar if c % 2 == 0 else nc.sync
                        eng.dma_start(
                            out=outt[c * P : (c + 1) * P, :], in_=o
                        )
                ht = new_ht

    nc.compile()
    return nc


def _maybe_reset_device():
    """Clear stale NRT state on the axon terminal left by a crashed prior
    process. Only safe/needed before this process initializes its jax
    backend, and must run in a subprocess (CDLL'ing the axon .so in-process
    conflicts with jax's own dlopen)."""
    try:
        import jax._src.xla_bridge as xb

        if getattr(xb, "_backends", None):
            return  # backend already live in this process; don't touch it
    except Exception:
        pass
    try:
        import subprocess

        subprocess.run(
            [
                sys.executable,
                "-c",
                "import ctypes; lib = ctypes.CDLL('/opt/axon/libaxon_pjrt.so'); "
                "lib.axon_reset.restype = ctypes.c_int64; lib.axon_reset()",
            ],
            timeout=60,
            capture_output=True,
        )
    except Exception:
        pass


def kernel(x, weight_blend, a0, b0, a1, b1, a2, b2):
    global LAST_RESULTS, _NC_CACHE
    _maybe_reset_device()
    x = np.asarray(x, dtype=np.float32)
    weight_blend = np.ascontiguousarray(np.asarray(weight_blend, dtype=np.float32))
    aT = []
    for a, (din, dout, _) in zip((a0, a1, a2), LAYERS):
        # [E, dout, din] -> aT [E, din, dout] -> [E, ni, 128, dout]
        # -> [E, 128, ni, dout] so each expert slab is one
        # partition-contiguous DMA
        at = (np.asarray(a, dtype=np.float32) * float(2.0**WEXP)).transpose(0, 2, 1)
        at = at.reshape(E, din // P, P, dout).transpose(0, 2, 1, 3)
        aT.append(np.ascontiguousarray(at.astype(np.float16)))
    bs = [
        np.ascontiguousarray(
            (np.asarray(b, dtype=np.float32) * float(2.0 ** (WEXP + ZEXP))).astype(
                np.float16
            )
        )
        for b in (b0, b1, b2)
    ]
    has_bias = any(np.any(b) for b in bs)

    if has_bias not in _NC_CACHE:
        _NC_CACHE[has_bias] = _build(has_bias)
    nc = _NC_CACHE[has_bias]

    in_maps = []
    for c in range(N_CORES):
        sl = slice(c * B_LOC, (c + 1) * B_LOC)
        wb_c = np.ascontiguousarray(weight_blend[:, sl]) * float(2.0**ZEXP)
        xt_c = x[sl].T.reshape(DIMS[0] // P, P, B_LOC).transpose(1, 0, 2)
        wbb_c = np.broadcast_to(wb_c[None, :, :], (P, E, B_LOC))
        m = {
            "xt": np.ascontiguousarray(xt_c.astype(np.float16)),
            "wbb": np.ascontiguousarray(wbb_c.astype(np.float16)),
            "a0t": aT[0],
            "a1t": aT[1],
            "a2t": aT[2],
        }
        if has_bias:
            m["wb"] = wb_c.astype(np.float16)
            m["b0"], m["b1"], m["b2"] = bs
        in_maps.append(m)

    trace = os.environ.get("BASS_KERNEL_TRACE") == "1"
    res = run_bass_kernel_spmd(
        nc, in_maps, core_ids=list(range(N_CORES)), trace=trace
    )
    LAST_RESULTS = res
    return np.concatenate(
        [np.asarray(r["outt"]).T.astype(np.float32) for r in res.results], axis=0
    )


# revision 9
# speedup vs baseline: 1.0391x; 1.0040x over previous
"""MoE soft-routing MLP kernel for 8 Trainium2 NeuronCores.

Reference computation (per layer l, weights a_l: [E, out, in], bias b_l: [E, out]):
    y_e = H @ a_e^T + b_e          # per-expert GEMM      [B, out]
    H'  = sum_e wb[e, :, None] * y_e                      [B, out]
    H'  = elu(H') for layers 0, 1

Distribution: data-parallel over batch B=4096 across 8 cores (B_loc=512).
Expert weights are replicated to every core; x and weight_blend are sharded
along batch.

Per-core algorithm (activations kept TRANSPOSED on chip: [feature, batch]):
    out[o, b] = sum_e sum_i aT_e[i, o] * (wb[e, b] * Ht[i, b])  + bias blend
  - each expert's contribution accumulates into one PSUM bank per output
    chunk: lhsT = aT_e[i-tile, o-chunk] (128x128 stationary), rhs = zt_e =
    Ht[i-tile] * bcast(wb[e, :]) (128x512 moving, fp16),
  - ELU+1 is evicted as relu(x) + min(exp(x), 1) into fp32 SBUF; the -1
    folds into the next layer's blend: zt = (h - 1) * wbb_e (one DVE op).

Matmuls are fp16 with fp32 PSUM accumulation. Weights are pre-scaled by 2^8
and blend weights by 2^6 on the host so fp16 products stay clear of the
subnormal range; the 2^-14 descale folds into the PSUM-eviction scales.
Measured end-to-end max rel-err vs the fp32 reference: ~5e-4.

Performance model (measured on hw):
  - PE is the wall: 1024 matmuls x 512 rows = 524288 cycles ~ 215us at
    2.4GHz. fp8 DoubleRow runs at the same rows/cycle (2x MACs via the
    in-pair contraction) so the ~8-bit precision this problem needs
    (hi+lo fp8 on both operands = 3 GEMM terms) would cost 1.5x fp16 —
    fp8 does not pay here. f32r matches fp16 rate but doubles DMA.
  - DMA: one big contiguous dma_start sustains ~370 GB/s (16 SDMA engines);
    partition-splitting a transfer HALVES bulk bandwidth and small chunks
    pay ~2us completion latency each. So weights stream as one 1-2MB
    dma_start per (layer, expert) slab, host-packed partition-contiguous
    ([128, ni*dout] rows). 34MB total ~ 95us, fully hidden under PE.
  - Startup: x^T (fp16, scalar queue) + wbb[0] (gpsimd) + first weight
    slab (sync) land in ~3us while junk matmuls warm the PE HAM clock
    gate (~3.4us of activity to reach 2.4GHz).
  - Tail: the final layer stores fp16 banks with a single dma_start each
    (no 8-way splitting), alternating scalar/sync queues.

The output is DMA'd out transposed ([512, 512] fp16 per core) and
un-transposed + upcast on the host.
"""

import os
import sys

if "/opt/trn_rl_repo" not in sys.path:
    sys.path.insert(0, "/opt/trn_rl_repo")

import numpy as np

import concourse.bass as bass  # noqa: F401  (bass must import before mybir use)
import concourse.mybir as mybir
import concourse.tile as tile
from concourse import bacc
from concourse.bass_utils import run_bass_kernel_spmd

F32 = mybir.dt.float32
F16 = mybir.dt.float16
AF = mybir.ActivationFunctionType
ALU = mybir.AluOpType

WEXP, ZEXP = 8, 6
DESCALE = float(2.0 ** -(WEXP + ZEXP))

B, E = 4096, 8
DIMS = [512, 1024, 1024, 512]
N_CORES = 8
B_LOC = B // N_CORES  # 512; also the matmul moving free-dim (max for 4-byte)
P = 128

# (in, out, apply_elu) per layer
LAYERS = [
    (DIMS[0], DIMS[1], True),
    (DIMS[1], DIMS[2], True),
    (DIMS[2], DIMS[3], False),
]

LAST_RESULTS = None  # BassKernelResults of the most recent run (for test.py)
_NC_CACHE = {}


def _build(has_bias):
    """Build the per-core module. has_bias=False (the case this problem's
    setup_inputs actually produces — all beta fills are zeros) drops the
    blended-bias matmuls and their beta/wb feeds entirely; each bank then
    closes on the last expert's product."""
    nc = bacc.Bacc(None, target_bir_lowering=False, debug=False)

    # Startup pack [128, 4608] fp16: wbb[0] (512) | x^T (4 j-tiles, 2048) |
    # expert-0 layer-0 weight j-tiles 0-1 (2048) — everything the first ~16
    # real matmuls need, landed by ONE dma (one completion receipt).
    ni0 = DIMS[0] // P
    PACK_COLS = B_LOC + ni0 * B_LOC + 2 * DIMS[1]
    packd = nc.dram_tensor("pack", [P, PACK_COLS], F16, kind="ExternalInput")
    # wbb host-packed [128, E, B_LOC] fp16 (partition-broadcast blend weights)
    wbbd = nc.dram_tensor("wbb", [P, E, B_LOC], F16, kind="ExternalInput")
    # weights host-packed per layer: [E, 128, ni, dout] fp16,
    # (e, p, j, o) = aT_l[e, j*128+p, o] — each expert slab is one
    # partition-contiguous [128, ni*dout] DMA.
    ats = [
        nc.dram_tensor(f"a{l}t", [E, P, din // P, dout], F16, kind="ExternalInput")
        for l, (din, dout, _) in enumerate(LAYERS)
    ]
    wb, betas = None, []
    if has_bias:
        wb = nc.dram_tensor("wb", [E, B_LOC], F16, kind="ExternalInput")
        betas = [
            nc.dram_tensor(f"b{l}", [E, dout], F16, kind="ExternalInput")
            for l, (_, dout, _) in enumerate(LAYERS)
        ]
    outt = nc.dram_tensor("outt", [DIMS[3], B_LOC], F16, kind="ExternalOutput")

    with tile.TileContext(nc) as tc:
        with (
            tc.tile_pool(name="xp", bufs=1) as xp,
            tc.tile_pool(name="htp", bufs=12) as htp,
            tc.tile_pool(name="ztp", bufs=16) as ztp,
            tc.tile_pool(name="wp", bufs=3) as wp,
            tc.tile_pool(name="wbbp", bufs=1) as wbbp,
            tc.tile_pool(name="consts", bufs=1) as consts,
            tc.tile_pool(name="betap", bufs=2) as betap,
            tc.tile_pool(name="tmp", bufs=4) as tmp,
            tc.tile_pool(name="psp", bufs=8, space="PSUM") as psp,
        ):
            # --- startup ---
            # PE warm-up: the HAM clock gate needs ~3.4us of sustained PE
            # activity to reach 2.4 GHz; junk matmuls also cover the first
            # input DMAs (~3us).
            junk = consts.tile([P, B_LOC], F16, tag="junk")
            nc.vector.memset(junk, 0.0)
            warm_ps = psp.tile([P, B_LOC], F32, tag="ps")
            for _ in range(12):
                nc.tensor.matmul(warm_ps, junk[:, :P], junk, start=True, stop=True)

            # Startup feeds share the single sync HWDGE queue with the
            # weight-slab stream, in priority order (wbb0, xt, slab0, wbbR,
            # slab1, ...): a second HWDGE ring gets starved for ~4us while
            # the first one has a packet backlog, so splitting queues
            # delays, not accelerates, the critical path. Separate tiles
            # for wbb[0] vs wbb[1:] so the first blend doesn't wait on the
            # bulk wbb transfer (Tile tracks dependencies per tile).
            pack_sb = xp.tile([P, PACK_COLS], F16, tag="pack")
            nc.sync.dma_start(out=pack_sb, in_=packd[:, :])
            wbbr_sb = wbbp.tile([P, E - 1, B_LOC], F16, tag="wbbr")
            wbb = [pack_sb[:, 0:B_LOC]] + [wbbr_sb[:, e - 1, :] for e in range(1, E)]
            # wb as [E, B_LOC] tile: rhs of the bias matmuls
            wb_all = None
            if has_bias:
                wb_all = consts.tile([E, B_LOC], F16, tag="wb_all")
                nc.gpsimd.dma_start(out=wb_all, in_=wb[:, :])

            # --- layers ---
            ht = [
                pack_sb[:, (1 + j) * B_LOC : (2 + j) * B_LOC] for j in range(ni0)
            ]
            for l, (din, dout, use_act) in enumerate(LAYERS):
                ni, no = din // P, dout // P
                beta_sb = None
                if has_bias:
                    beta_sb = betap.tile([E, dout], F16, tag="beta")
                    nc.gpsimd.dma_start(out=beta_sb, in_=betas[l][:, :])

                psums = [psp.tile([P, B_LOC], F32, tag="ps", name="ps") for _ in range(no)]

                # one partition-contiguous weight slab per expert on the
                # sync queue (~370GB/s; stays well ahead of PE consumption)
                def load_slab(e):
                    t = wp.tile([P, ni, dout], F16, tag=f"w{l}")
                    nc.sync.dma_start(out=t, in_=ats[l][e])
                    return t

                def wslice(e, j, c):
                    if l == 0 and e == 0:
                        if j < 2:
                            base = (1 + ni0) * B_LOC + j * dout
                            return pack_sb[:, base + c * P : base + (c + 1) * P]
                        return slabs[0][:, j - 2, c * P : (c + 1) * P]
                    return slabs[e][:, j, c * P : (c + 1) * P]

                if l == 0:
                    # expert-0 j-tiles 2..ni-1 (0-1 ride in the startup
                    # pack), then bulk blend weights (needed from expert 1)
                    t0 = wp.tile([P, ni - 2, dout], F16, tag="w0tail")
                    nc.sync.dma_start(out=t0, in_=ats[0][0, :, 2:ni, :])
                    slabs = {0: t0}
                    nc.sync.dma_start(out=wbbr_sb, in_=wbbd[:, 1:E, :])
                else:
                    slabs = {0: load_slab(0)}
                slabs[1] = load_slab(1)

                # experts 0..E-2 j-outer (consumes ht tiles as the previous
                # layer produces them; first expert opens each bank)
                for e in range(E - 1):
                    if e + 2 < E:
                        slabs[e + 2] = load_slab(e + 2)
                    for j in range(ni):
                        zt = ztp.tile([P, B_LOC], F16, tag="zt")
                        if l == 0:
                            nc.vector.tensor_mul(zt, ht[j], wbb[e])
                        else:
                            # ht holds elu(x)+1; fold the -1 into the blend
                            nc.vector.scalar_tensor_tensor(
                                zt, ht[j], -1.0, wbb[e], ALU.add, ALU.mult
                            )
                        for c in range(no):
                            nc.tensor.matmul(
                                psums[c],
                                wslice(e, j, c),
                                zt,
                                start=(e == 0 and j == 0),
                                stop=False,
                            )
                # last expert runs c-outer (bank-by-bank) so bank closures —
                # and therefore evictions, next-layer bank reuse, and the
                # final output stores — spread across the last ~ni*no
                # matmuls instead of clustering after the end.
                e = E - 1
                zts = []
                for j in range(ni):
                    zt = ztp.tile([P, B_LOC], F16, tag="zt")
                    if l == 0:
                        nc.vector.tensor_mul(zt, ht[j], wbb[e])
                    else:
                        nc.vector.scalar_tensor_tensor(
                            zt, ht[j], -1.0, wbb[e], ALU.add, ALU.mult
                        )
                    zts.append(zt)
                new_ht = []
                for c in range(no):
                    for j in range(ni):
                        nc.tensor.matmul(
                            psums[c],
                            wslice(e, j, c),
                            zts[j],
                            start=False,
                            stop=(not has_bias and j == ni - 1),
                        )
                    if has_bias:
                        nc.tensor.matmul(
                            psums[c],
                            beta_sb[:, c * P : (c + 1) * P],
                            wb_all,
                            start=False,
                            stop=True,
                        )

                    # evict bank c as soon as it closes:
                    # elu(x)+1 into fp32 ht for layers 0/1, fp16 DMA out for
                    # layer 2
                    if use_act:
                        r = tmp.tile([P, B_LOC], F32, tag="relu")
                        x = tmp.tile([P, B_LOC], F32, tag="expz")
                        h = htp.tile([P, B_LOC], F32, tag="ht")
                        nc.scalar.activation(r, psums[c], AF.Relu, scale=DESCALE)
                        nc.scalar.activation(x, psums[c], AF.Exp, scale=DESCALE)
                        # h = min(x, 1) + r  ( = elu + 1 )
                        nc.vector.scalar_tensor_tensor(h, x, 1.0, r, ALU.min, ALU.add)
                        new_ht.append(h)
                    else:
                        # descale-copy split across ACT and DVE (engine time
                        # scales with the free dim; both engines are
                        # otherwise idle at kernel end), then one unsplit
                        # 128KB store per bank on an alternating HWDGE queue
                        o = tmp.tile([P, B_LOC], F16, tag="out")
                        nc.scalar.activation(
                            o[: P // 2, :], psums[c][: P // 2, :], AF.Copy,
                            scale=DESCALE,
                        )
                        nc.vector.tensor_scalar_mul(
                            o[P // 2 :, :], psums[c][P // 2 :, :], DESCALE
                        )
                        eng = nc.scalar if c % 2 == 0 else nc.sync
                        eng.dma_start(
                            out=outt[c * P : (c + 1) * P, :], in_=o
                        )
                ht = new_ht

    nc.compile()
    return nc


def _maybe_reset_device():
    """Clear stale NRT state on the axon terminal left by a crashed prior
    process. Only safe/needed before this process initializes its jax
    backend, and must run in a subprocess (CDLL'ing the axon .so in-process
    conflicts with jax's own dlopen)."""
    try:
        import jax._src.xla_bridge as xb

        if getattr(xb, "_backends", None):
            return  # backend already live in this process; don't touch it
    except Exception:
        pass
    try:
        import subprocess

        subprocess.run(
            [
                sys.executable,
                "-c",
                "import ctypes; lib = ctypes.CDLL('/opt/axon/libaxon_pjrt.so'); "
                "lib.axon_reset.restype = ctypes.c_int64; lib.axon_reset()",
            ],
            timeout=60,
            capture_output=True,
        )
    except Exception:
        pass


def kernel(x, weight_blend, a0, b0, a1, b1, a2, b2):
    global LAST_RESULTS, _NC_CACHE
    _maybe_reset_device()
    x = np.asarray(x, dtype=np.float32)
    weight_blend = np.ascontiguousarray(np.asarray(weight_blend, dtype=np.float32))
    aT = []
    for a, (din, dout, _) in zip((a0, a1, a2), LAYERS):
        # [E, dout, din] -> aT [E, din, dout] -> [E, ni, 128, dout]
        # -> [E, 128, ni, dout] so each expert slab is one
        # partition-contiguous DMA
        at = (np.asarray(a, dtype=np.float32) * float(2.0**WEXP)).transpose(0, 2, 1)
        at = at.reshape(E, din // P, P, dout).transpose(0, 2, 1, 3)
        aT.append(np.ascontiguousarray(at.astype(np.float16)))
    bs = [
        np.ascontiguousarray(
            (np.asarray(b, dtype=np.float32) * float(2.0 ** (WEXP + ZEXP))).astype(
                np.float16
            )
        )
        for b in (b0, b1, b2)
    ]
    has_bias = any(np.any(b) for b in bs)

    if has_bias not in _NC_CACHE:
        _NC_CACHE[has_bias] = _build(has_bias)
    nc = _NC_CACHE[has_bias]

    in_maps = []
    for c in range(N_CORES):
        sl = slice(c * B_LOC, (c + 1) * B_LOC)
        wb_c = np.ascontiguousarray(weight_blend[:, sl]) * float(2.0**ZEXP)
        xt_c = x[sl].T.reshape(DIMS[0] // P, P, B_LOC).transpose(1, 0, 2)
        wbb_c = np.broadcast_to(wb_c[None, :, :], (P, E, B_LOC))
        pack = np.concatenate(
            [
                np.broadcast_to(wb_c[0][None, :], (P, B_LOC)).astype(np.float16),
                xt_c.astype(np.float16).reshape(P, DIMS[0] // P * B_LOC),
                aT[0][0][:, 0:2, :].reshape(P, 2 * DIMS[1]),
            ],
            axis=1,
        )
        m = {
            "pack": np.ascontiguousarray(pack),
            "wbb": np.ascontiguousarray(wbb_c.astype(np.float16)),
            "a0t": aT[0],
            "a1t": aT[1],
            "a2t": aT[2],
        }
        if has_bias:
            m["wb"] = wb_c.astype(np.float16)
            m["b0"], m["b1"], m["b2"] = bs
        in_maps.append(m)

    trace = os.environ.get("BASS_KERNEL_TRACE") == "1"
    res = run_bass_kernel_spmd(
        nc, in_maps, core_ids=list(range(N_CORES)), trace=trace
    )
    LAST_RESULTS = res
    return np.concatenate(
        [np.asarray(r["outt"]).T.astype(np.float32) for r in res.results], axis=0
    )
